# revision 4
# baseline (speedup 1.0000x reference)
"""Trainium2 Bass kernel v2 for nn_Cell_82729660056407 (DARTS-style 1D cell).

Same sharding/layout as v1 (Toeplitz, 8-way batch-parallel), restructured so
the TensorEngine does all linear work (PSUM-accumulated scaled-band matmuls)
and round-A BN stats come from autocorrelation lags instead of per-conv
square passes.
"""
import sys, os
sys.path.insert(0, "/opt/trn_rl_repo")
os.environ.setdefault("JAX_PLATFORMS", "cpu")

import numpy as np
from contextlib import ExitStack

import concourse.bass as bass
import concourse.bacc as bacc
import concourse.mybir as mybir
import concourse.tile as tile
import concourse.bass_isa as bass_isa
from concourse import library_config

# ---------------- constants ----------------
B, L = 64, 32768
NCORE = 8
BL = B // NCORE          # rows per core = 8
P = 128
HALO = 4
UU = 120
NBLK = (L + UU - 1) // UU   # 274
F = NBLK * BL               # 2192
NG = B * L
EPS = 1e-5
STEPS = 4
F16 = mybir.dt.float16
F32 = mybir.dt.float32
AL = mybir.AluOpType
AF = mybir.ActivationFunctionType

LAGS = [0, 1, 2, 3, 4, 6, 8]
NLAG = len(LAGS)            # 7
CHUNKS = [(0, 512), (512, 1024), (1024, 1536), (1536, 2048), (2048, 2192)]
HALVES = [(0, 1096), (1096, 2192)]
CH0 = [(0, 512), (512, 1024), (1024, 1096)]   # chunk layout inside a half

# stats subsampling: first SUB f-columns = blocks 0..136 -> 131520 valid
# elements per core (of 262144); scale factors fold into host consts.
SUB = 512
NSAMP = (SUB // BL) * UU * BL          # 131520
SCALE_RHO = (L * BL) / NSAMP
NGB = NCORE * NSAMP                    # round-B sumsq divisor

# band indices
def BD_DYN(i, k):  # k: 0 s3a, 1 s5a, 2 d3, 3 d5, 4 s3b, 5 s5b
    return 6 * i + k
def _wrow(i, j):
    return i * (i + 1) // 2 + j
def BD_STATIC(i, j): return 24 + _wrow(i, j)
def BD_EDGEL(i, j):  return 34 + _wrow(i, j)
def BD_EDGER(i, j):  return 44 + _wrow(i, j)
def BD_MP(i, j):     return 54 + _wrow(i, j)
BD_I = 64
def BD_SHIFT(idx): return 65 + idx      # idx over LAGS[1:]
BD_SHIFTM1 = 71                         # shift by -1 (for maxpool)
BD_NINJ = 72                            # rows 0/1: -30000 at m=4 / m=11
NBAND = 73
NLAGX = NLAG + 2                        # + wrap slots for lags 6, 8


def _band(taps, dil):
    k = len(taps)
    pad = dil * (k - 1) // 2
    bd = np.zeros((P, P), np.float64)
    for t in range(k):
        for m in range(P):
            kk = m + dil * t - pad
            if 0 <= kk < P:
                bd[kk, m] += taps[t]
    return bd


def _sgn(v):
    s = float(np.sign(float(v)))
    return s if s != 0.0 else 1.0


def _eps(pw):
    pw = float(pw)
    if pw == 0.0:
        return np.float32(1e30)
    return np.float32(EPS / (pw * pw))


def make_host_consts(inputs):
    w = np.asarray(inputs["weights"], np.float64)   # [10, 8]
    taps = {}
    for i in range(STEPS):
        taps[(i, 0)] = (np.asarray(inputs["sep3_dw1"][i], np.float64), 1)
        taps[(i, 1)] = (np.asarray(inputs["sep5_dw1"][i], np.float64), 1)
        taps[(i, 2)] = (np.asarray(inputs["dil3_dw"][i], np.float64), 2)
        taps[(i, 3)] = (np.asarray(inputs["dil5_dw"][i], np.float64), 2)
        taps[(i, 4)] = (np.asarray(inputs["sep3_dw2"][i], np.float64), 1)
        taps[(i, 5)] = (np.asarray(inputs["sep5_dw2"][i], np.float64), 1)

    bands = np.zeros((NBAND, P, P), np.float64)
    for i in range(STEPS):
        for k in range(6):
            bands[BD_DYN(i, k)] = _band(*taps[(i, k)])
    for i in range(STEPS):
        for j in range(i + 1):
            ww = w[_wrow(i, j)]
            bands[BD_STATIC(i, j)] = (_band([ww[2] / 3.0] * 3, 1)
                                      + ww[3] * np.eye(P))
            bl = np.zeros((P, P)); bl[4, 4] = ww[2] / 6.0; bl[5, 4] = ww[2] / 6.0
            bands[BD_EDGEL(i, j)] = bl
            br = np.zeros((P, P)); br[10, 11] = ww[2] / 6.0; br[11, 11] = ww[2] / 6.0
            bands[BD_EDGER(i, j)] = br
            bands[BD_MP(i, j)] = ww[1] * np.eye(P)
    bands[BD_I] = np.eye(P)
    for idx, delta in enumerate(LAGS[1:]):
        sh = np.zeros((P, P))
        for m in range(P - delta):
            sh[m + delta, m] = 1.0
        bands[BD_SHIFT(idx)] = sh
    shm = np.zeros((P, P))
    for m in range(1, P):
        shm[m - 1, m] = 1.0
    bands[BD_SHIFTM1] = shm
    nj = np.zeros((P, P))
    nj[0, 4] = -30000.0      # hm inject at partition 4 (head)
    nj[32, 11] = -30000.0    # hp inject at partition 11 (tail); row 32 for
    bands[BD_NINJ] = nj      # the lhsT base-partition alignment rule
    bands16 = bands.astype(np.float16)
    bands_dev = np.ascontiguousarray(
        bands16.transpose(1, 0, 2)).reshape(P, NBAND * P)

    # fp16-rounded taps for stats consistency with actual matmuls
    b16 = bands16.astype(np.float64)

    # Cmat_all [128, 16] fp32: col 4i+op = c-vector of band (i, op<4)
    mask_m = np.zeros(P); mask_m[HALO:P - HALO] = 1.0
    cmat = np.zeros((P, 16), np.float64)
    for i in range(STEPS):
        for op in range(4):
            cmat[:, 4 * i + op] = b16[BD_DYN(i, op)] @ mask_m
    # Cmat2 [128, 8] fp32: col 2i+op2 = c-vector of band (i, 4+op2)
    cmat2 = np.zeros((P, 8), np.float64)
    K2 = np.zeros((STEPS, 2))
    for i in range(STEPS):
        for op2 in range(2):
            cv = b16[BD_DYN(i, 4 + op2)] @ mask_m
            cmat2[:, 2 * i + op2] = cv
            K2[i, op2] = 8.0 * NCORE * cv[16:].sum()

    # autocorr combine coefficients: coefA[i][op][lag_idx]
    coefA = np.zeros((STEPS, 4, NLAG))
    for i in range(STEPS):
        for op in range(4):
            tp, dil = taps[(i, op)]
            tp = tp.astype(np.float16).astype(np.float64)  # fp16-rounded taps
            k = len(tp)
            for t in range(k):
                for t2 in range(k):
                    d = abs(dil * (t2 - t))
                    coefA[i, op, LAGS.index(d)] += SCALE_RHO * tp[t] * tp[t2]

    # per-step replicated const tiles (fp32), op-major (col = op*ns + j):
    # CArep_i [P, 4*7*ns]; evA_i/cvA_i [P, 4*ns]; ev2_i/cv2_i [P, 2*ns]
    cons = {}
    for i in range(STEPS):
        ns = i + 1
        ca = np.zeros((4, ns, NLAGX), np.float32)
        ev = np.zeros((4, ns), np.float32)
        cv = np.zeros((4, ns), np.float32)
        ev2 = np.zeros((2, ns), np.float32)
        cv2 = np.zeros((2, ns), np.float32)
        for j in range(ns):
            ww = w[_wrow(i, j)]
            for op in range(4):
                ca[op, j, 0:NLAG] = coefA[i, op]
                ca[op, j, NLAG + 0] = coefA[i, op, LAGS.index(6)]
                ca[op, j, NLAG + 1] = coefA[i, op, LAGS.index(8)]
            ev[0, j] = _eps(inputs["sep3_pw1"][i]); cv[0, j] = _sgn(inputs["sep3_pw1"][i])
            ev[1, j] = _eps(inputs["sep5_pw1"][i]); cv[1, j] = _sgn(inputs["sep5_pw1"][i])
            ev[2, j] = _eps(inputs["dil3_pw"][i]);  cv[2, j] = ww[6] * _sgn(inputs["dil3_pw"][i])
            ev[3, j] = _eps(inputs["dil5_pw"][i]);  cv[3, j] = ww[7] * _sgn(inputs["dil5_pw"][i])
            ev2[0, j] = _eps(inputs["sep3_pw2"][i]); cv2[0, j] = ww[4] * _sgn(inputs["sep3_pw2"][i])
            ev2[1, j] = _eps(inputs["sep5_pw2"][i]); cv2[1, j] = ww[5] * _sgn(inputs["sep5_pw2"][i])
        cons[("ca", i)] = ca.reshape(-1)
        cons[("ev", i)] = ev.reshape(-1)
        cons[("cv", i)] = cv.reshape(-1)
        cons[("ev2", i)] = ev2.reshape(-1)
        cons[("cv2", i)] = cv2.reshape(-1)

    # pack all fp32 consts into one [P, NC] tensor (replicated rows), plus
    # partition-major blocks cmat/cmat2/mask appended as extra columns groups.
    blocks = []
    offs = {}
    cur = 0
    for key, vec in cons.items():
        offs[key] = cur
        blocks.append(np.broadcast_to(vec.astype(np.float32), (P, len(vec))))
        cur += len(vec)
    offs["cmat"] = cur
    blocks.append(cmat.astype(np.float32)); cur += 16
    offs["cmat2"] = cur
    blocks.append(cmat2.astype(np.float32)); cur += 8
    offs["mask"] = cur
    mk = np.zeros((P, 1), np.float32); mk[HALO:P - HALO] = 1.0
    blocks.append(mk); cur += 1
    consts = np.concatenate(blocks, axis=1).astype(np.float32)

    return dict(bands=bands_dev, consts=consts, offs=offs, ncols=cur,
                w=w, K2=K2)


def toeplitz_shard(x):
    from numpy.lib.stride_tricks import as_strided
    shards = []
    padlen = (NBLK - 1) * UU + P
    for c in range(NCORE):
        xr = np.ascontiguousarray(x[c * BL:(c + 1) * BL], np.float32)
        xpad = np.zeros((BL, padlen), np.float32)
        xpad[:, HALO:HALO + L] = xr
        v = as_strided(xpad, shape=(BL, NBLK, P),
                       strides=(xpad.strides[0], UU * 4, 4))
        xt = np.ascontiguousarray(v.transpose(2, 1, 0)).reshape(P, F)
        shards.append(xt.astype(np.float16))
    return shards


def untoeplitz(out_t):
    v = out_t.reshape(P, NBLK, BL)[HALO:HALO + UU]
    o = v.transpose(2, 1, 0).reshape(BL, NBLK * UU)
    return o[:, :L]


def build_program(hc, dbg_steps=STEPS):
    offs = hc["offs"]
    K2 = hc["K2"]
    nc = bacc.Bacc("TRN2", target_bir_lowering=False, debug=False,
                   num_devices=NCORE)
    xt_d = nc.dram_tensor("xt", [P, F], F16, kind="ExternalInput")
    bands_d = nc.dram_tensor("bands", [P, NBAND * P], F16, kind="ExternalInput")
    consts_d = nc.dram_tensor("consts", [P, hc["ncols"]], F32, kind="ExternalInput")
    out_d = nc.dram_tensor("out", [P, F], F32, kind="ExternalOutput")
    dbgd = nc.dram_tensor("dbg", [P, 200], F32, kind="ExternalOutput")

    ctx = ExitStack()
    with tile.TileContext(nc) as tc:
        sbp = ctx.enter_context(tc.tile_pool(name="sbp", bufs=1))       # persistent
        r1p = ctx.enter_context(tc.tile_pool(name="r1p", bufs=8))       # r1 tiles
        trp = ctx.enter_context(tc.tile_pool(name="trp", bufs=2))       # trash
        alg = ctx.enter_context(tc.tile_pool(name="alg", bufs=2))       # algebra
    # accum partial tiles
        acc = ctx.enter_context(tc.tile_pool(name="acc", bufs=2))
        bsp = ctx.enter_context(tc.tile_pool(name="bsp", bufs=4))       # scaled bands
        snp = ctx.enter_context(tc.tile_pool(name="snp", bufs=1, space="PSUM"))
        cps = ctx.enter_context(tc.tile_pool(name="cps", bufs=1, space="PSUM"))
        drp = ctx.enter_context(tc.tile_pool(name="drp", bufs=2, space="DRAM"))

        # ---- load constants ----
        bsb = sbp.tile([P, NBAND * P], F16, name="bsb", tag="bsb")
        def band_ap(k):
            return bsb[:, k * P:(k + 1) * P]

        csb = sbp.tile([P, hc["ncols"]], F32, name="csb", tag="csb")
        nc.sync.dma_start(csb[:], consts_d.ap())
        def cview(key, n):
            o = offs[key]
            return csb[:, o:o + n]
        mask_ap = csb[:, offs["mask"]:offs["mask"] + 1]

        zeros_sb = sbp.tile([P, 16], F16, name="zeros_sb", tag="zeros_sb")
        nc.vector.memset(zeros_sb[:], 0.0)
        ones_sb = sbp.tile([33, 16], F16, name="ones_sb", tag="ones_sb")
        nc.vector.memset(ones_sb[:], 1.0)

        # ---- persistent state tiles ----
        h = [sbp.tile([P, F], F16, name=f"h{s}", tag=f"h{s}")
             for s in range(STEPS)]        # h[0]=x .. h[3]; no h4
        r = [sbp.tile([P, F], F16, name=f"r{s}", tag=f"r{s}")
             for s in range(STEPS)]
        mp = [sbp.tile([P, F], F16, name=f"mp{s}", tag=f"mp{s}")
              for s in range(STEPS)]
        rho_g = sbp.tile([P, 4 * NLAGX], F32, name="rho_g", tag="rho_g")
        stash = sbp.tile([8, 8], F16, name="stash", tag="stash")
        wrpA = sbp.tile([4, SUB], F16, name="wrpA", tag="wrpA")
        wrpB = sbp.tile([4, SUB], F16, name="wrpB", tag="wrpB")
        wrpC = sbp.tile([2, SUB], F16, name="wrpC", tag="wrpC")
        wrpD = sbp.tile([2, SUB], F16, name="wrpD", tag="wrpD")
        s0a_g = sbp.tile([P, 4 * 16], F32, name="s0a_g", tag="s0a_g")

        nc.sync.dma_start(h[0][:], xt_d.ap())
        # shift/inject bands first: state-0 birth needs them immediately
        nc.sync.dma_start(bsb[:, 65 * P:NBAND * P], bands_d.ap()[:, 65 * P:NBAND * P])
        nc.sync.dma_start(bsb[:, 0:65 * P], bands_d.ap()[:, 0:65 * P])

        def emit_maxpool(s):
            # hp/hm shifts via PE shift bands into PSUM; -inf edge injects
            # via rank-1 accumulate matmuls; maxes on DVE (one PSUM operand)
            mq = snp.tile([P, F], F32, name=f"mq{s}", tag="sps")
            for ci, (c0, c1) in enumerate(CHUNKS):
                W = c1 - c0
                b0 = (ci % 2) * 1024
                hp_r = mq[:, b0:b0 + W]
                hm_r = mq[:, b0 + 512:b0 + 512 + W]
                last = (ci == len(CHUNKS) - 1)
                nc.tensor.matmul(hp_r, band_ap(BD_SHIFT(0)), h[s][:, c0:c1],
                                 start=True, stop=not last)
                if last:   # -inf at position L-1 (partition 11, last 8 cols)
                    nc.tensor.matmul(mq[:, b0 + W - 8:b0 + W],
                                     band_ap(BD_NINJ)[32:33, :], ones_sb[32:33, 0:8],
                                     start=False, stop=True)
                nc.tensor.matmul(hm_r, band_ap(BD_SHIFTM1), h[s][:, c0:c1],
                                 start=True, stop=ci != 0)
                if ci == 0:  # -inf at position 0 (partition 4, first 8 cols)
                    nc.tensor.matmul(mq[:, b0 + 512:b0 + 520],
                                     band_ap(BD_NINJ)[0:1, :], ones_sb[0:1, 0:8],
                                     start=False, stop=True)
                m1 = trp.tile([P, 512], F16, name="m1", tag="m1")
                nc.vector.tensor_tensor(out=m1[:, 0:W], in0=h[s][:, c0:c1],
                                        in1=hp_r, op=AL.max)
                nc.vector.tensor_tensor(out=mp[s][:, c0:c1], in0=m1[:, 0:W],
                                        in1=hm_r, op=AL.max)

        def state_birth(s):
            """r[s]=relu(h[s]) w/ rowsum accum; shift tiles; rho partials
            (subsampled to cols 0:SUB); sums row-matmul; one AllReduce;
            broadcast into rho_g/s0a_g."""
            SUBE = SUB + 8
            Rta = acc.tile([P, 2], F32, name=f"Rta{s}", tag="Rta")
            # part1 (cols 0:SUBE) on DVE -- unblocks rho fast
            nc.vector.tensor_scalar(out=r[s][:, 0:SUBE], in0=h[s][:, 0:SUBE],
                                    scalar1=0.0, scalar2=None, op0=AL.max,
                                    op1=AL.add, accum_out=Rta[:, 0:1])
            # shifted copies via PE shift-band matmuls into spare psum banks
            shp = snp.tile([P, F], F32, name=f"shp{s}", tag="sps")
            shc = cps.tile([P, 1096], F32, name=f"shc{s}", tag="cps")
            shifts = []
            for idx in range(4):
                c0 = idx * 512
                nc.tensor.matmul(shp[:, c0:c0 + SUB], band_ap(BD_SHIFT(idx)),
                                 r[s][:, 0:SUB], start=True, stop=True)
                shifts.append(shp[:, c0:c0 + SUB])
            for idx in range(4, 6):
                c0 = (idx - 4) * 512
                nc.tensor.matmul(shc[:, c0:c0 + SUB], band_ap(BD_SHIFT(idx)),
                                 r[s][:, 0:SUB], start=True, stop=True)
                shifts.append(shc[:, c0:c0 + SUB])
            # wrap rows for lags 6, 8 (tiny tiles, equal base partitions)
            nc.sync.dma_start(wrpC[0:2, :], r[s][122:124, 0:SUB])
            nc.sync.dma_start(wrpD[0:2, :], r[s][8:10, 8:SUB + 8])
            nc.sync.dma_start(wrpA[0:4, :], r[s][120:124, 0:SUB])
            nc.sync.dma_start(wrpB[0:4, :], r[s][8:12, 8:SUB + 8])
            rhop = acc.tile([P, NLAGX], F32, name=f"rhop{s}", tag="rhop")
            nc.vector.memset(rhop[:, NLAG:NLAGX], 0.0)
            # lag 0 on ACT: Square(mask*r), accum
            tr0 = trp.tile([P, SUB], F16, name="tr0", tag="trash")
            nc.scalar.activation(tr0[:], r[s][:, 0:SUB], AF.Square, scale=mask_ap,
                                 accum_out=rhop[:, 0:1])
            # part2 of relu on ACT
            nc.scalar.activation(r[s][:, SUBE:F], h[s][:, SUBE:F], AF.Relu,
                                 accum_out=Rta[:, 1:2])
            # lags >0 on DVE: (r*mask)*shift_psum
            for idx in range(1, NLAG):
                trl = trp.tile([P, SUB], F16, name=f"trl{idx}", tag="trash")
                nc.vector.scalar_tensor_tensor(
                    out=trl[:], in0=r[s][:, 0:SUB], scalar=mask_ap,
                    in1=shifts[idx - 1],
                    op0=AL.mult, op1=AL.mult, accum_out=rhop[:, idx:idx + 1])
            # wrap products (tiny)
            trw = trp.tile([P, SUB], F16, name="trw", tag="trash")
            nc.vector.scalar_tensor_tensor(
                out=trw[0:2, :], in0=wrpC[0:2, :], scalar=1.0,
                in1=wrpD[0:2, :], op0=AL.mult, op1=AL.mult,
                accum_out=rhop[0:2, NLAG:NLAG + 1])
            trw2 = trp.tile([P, SUB], F16, name="trw2", tag="trash")
            nc.vector.scalar_tensor_tensor(
                out=trw2[0:4, :], in0=wrpA[0:4, :], scalar=1.0,
                in1=wrpB[0:4, :], op0=AL.mult, op1=AL.mult,
                accum_out=rhop[0:4, NLAG + 1:NLAG + 2])
            Rt = acc.tile([P, 1], F32, name=f"Rt{s}", tag="Rt")
            nc.vector.tensor_tensor(out=Rt[:], in0=Rta[:, 0:1], in1=Rta[:, 1:2],
                                    op=AL.add)
            red = acc.tile([P, NLAGX], F32, name=f"red{s}", tag="red")
            nc.gpsimd.partition_all_reduce(red[:], rhop[:], channels=P,
                                           reduce_op=bass_isa.ReduceOp.add)
            # sums row: [1,16] = Rt^T @ Cmat_all
            psr = cps.tile([1, 16], F32, name=f"psr{s}", tag="cps")
            nc.tensor.matmul(psr[:, 0:16], Rt[:], cview("cmat", 16),
                             start=True, stop=True)
            srow = acc.tile([1, 16 + NLAGX], F32, name=f"srow{s}", tag="srow")
            nc.vector.tensor_copy(srow[0:1, 0:16], psr[:])
            nc.vector.tensor_copy(srow[0:1, 16:16 + NLAGX], red[0:1, 0:NLAGX])
            ar_in = drp.tile([1, 16 + NLAGX], F32, name=f"arA{s}", tag="arA")
            ar_out = drp.tile([1, 16 + NLAGX], F32, name=f"aroA{s}", tag="aroA",
                              addr_space="Shared")
            nc.gpsimd.dma_start(ar_in[:], srow[:])
            nc.gpsimd.collective_compute(
                "AllReduce", AL.add, replica_groups=[list(range(NCORE))],
                ins=[ar_in.opt()], outs=[ar_out.opt()])
            back = acc.tile([1, 16 + NLAGX], F32, name=f"back{s}", tag="back")
            nc.gpsimd.dma_start(back[:], ar_out[:])
            nc.gpsimd.partition_broadcast(s0a_g[:, 16 * s:16 * s + 16],
                                          back[0:1, 0:16], channels=P)
            nc.gpsimd.partition_broadcast(rho_g[:, NLAGX * s:NLAGX * (s + 1)],
                                          back[0:1, 16:16 + NLAGX], channels=P)

        state_birth(0)
        emit_maxpool(0)

        # ================= step loop =================
        for i in range(dbg_steps):
            ns = i + 1

            # ---- algebra A: aA,bA [P, 4*ns] op-major (col = op*ns+j) ----
            nu = 4 * ns
            S0 = alg.tile([P, nu], F32, name=f"S0A{i}", tag="S0")
            for op in range(4):
                nc.vector.tensor_copy(
                    S0[:, op * ns:(op + 1) * ns],
                    s0a_g[:, 4 * i + op: min(4 * i + op + 16 * ns, 64): 16])
            SQ = alg.tile([P, nu], F32, name=f"SQA{i}", tag="SQ")
            ca = cview(("ca", i), 4 * ns * NLAGX)
            for op in range(4):
                tmp = alg.tile([P, ns * NLAGX], F32, name=f"catmp{i}_{op}", tag="catmp")
                nc.vector.tensor_tensor(out=tmp[:], in0=rho_g[:, 0:ns * NLAGX],
                                        in1=ca[:, op * ns * NLAGX:(op + 1) * ns * NLAGX],
                                        op=AL.mult)
                nc.vector.tensor_reduce(
                    out=SQ[:, op * ns:(op + 1) * ns],
                    in_=tmp[:, 0:ns * NLAGX].rearrange("p (j l) -> p j l", j=ns, l=NLAGX),
                    axis=mybir.AxisListType.X, op=AL.add)

            def bn_algebra(S0t, SQt, evv, cvv, n, rnd, ng_sq=NG):
                nmusq = alg.tile([P, n], F32, name=f"nmusq{rnd}", tag="nmusq")
                nc.vector.scalar_tensor_tensor(out=nmusq[:], in0=S0t[:],
                                               scalar=-1.0 / (NG * NG), in1=S0t[:],
                                               op0=AL.mult, op1=AL.mult)
                var = alg.tile([P, n], F32, name=f"var{rnd}", tag="var")
                nc.vector.scalar_tensor_tensor(out=var[:], in0=SQt[:],
                                               scalar=1.0 / ng_sq, in1=nmusq[:],
                                               op0=AL.mult, op1=AL.add)
                vare = alg.tile([P, n], F32, name=f"vare{rnd}", tag="vare")
                nc.vector.tensor_tensor(out=vare[:], in0=var[:], in1=evv, op=AL.add)
                sg = alg.tile([P, n], F32, name=f"sg{rnd}", tag="sg")
                nc.scalar.sqrt(sg[:], vare[:])
                rstd = alg.tile([P, n], F32, name=f"rstd{rnd}", tag="rstd")
                nc.vector.reciprocal(rstd[:], sg[:])
                a = alg.tile([P, n], F32, name=f"a{rnd}", tag=f"a{rnd}")
                nc.vector.tensor_tensor(out=a[:], in0=rstd[:], in1=cvv, op=AL.mult)
                b = alg.tile([P, n], F32, name=f"b{rnd}", tag=f"b{rnd}")
                nc.vector.scalar_tensor_tensor(out=b[:], in0=S0t[:],
                                               scalar=-1.0 / NG, in1=a[:],
                                               op0=AL.mult, op1=AL.mult)
                return a, b, None

            aA, bA, _ = bn_algebra(S0, SQ, cview(("ev", i), nu),
                                   cview(("cv", i), nu), nu, f"A{i}")

            # ---- s_new PSUM accumulation ----
            sps = snp.tile([P, F], F32, name=f"sps{i}", tag="sps")
            # per-chunk matmul totals so start/stop flags close each bank's group
            last_full = (i == dbg_steps - 1) and (dbg_steps == STEPS)
            tot = [5 * ns + (3 if last_full else 0)] * len(CHUNKS)
            tot[0] += ns          # left edge fixes land in chunk 0
            tot[-1] += ns         # right edge fixes land in last chunk
            cnt = [0] * len(CHUNKS)

            def mm_chunk(ci, c0, c1, bnd_ap, rhs_ap):
                nc.tensor.matmul(sps[:, c0:c1], bnd_ap, rhs_ap,
                                 start=(cnt[ci] == 0),
                                 stop=(cnt[ci] == tot[ci] - 1))
                cnt[ci] += 1

            def mm_full(bnd_ap, rhs):
                for ci, (c0, c1) in enumerate(CHUNKS):
                    mm_chunk(ci, c0, c1, bnd_ap, rhs[:, c0:c1])

            # statics + mp terms: stat-free PE work upfront (covers ccA wait)
            for j in range(ns):
                mm_full(band_ap(BD_STATIC(i, j)), h[j])
                mm_chunk(0, 0, 8, band_ap(BD_EDGEL(i, j)), h[j][:, 0:8])
                mm_chunk(len(CHUNKS) - 1, 2184, 2192, band_ap(BD_EDGER(i, j)),
                         h[j][:, 2184:2192])
            for j in range(ns):
                mm_full(band_ap(BD_MP(i, j)), mp[j])
            # dil chunks become fillers, interleaved with the u1/u2 conv
            # phase (keeps PE busy while ACT evacuates)
            fillers = []
            for j in range(ns):
                t5 = bsp.tile([P, P], F16, name=f"t5_{i}_{j}", tag="bsc")
                nc.vector.tensor_scalar(out=t5[:], in0=band_ap(BD_DYN(i, 3)),
                                        scalar1=aA[:, 3 * ns + j:3 * ns + j + 1],
                                        scalar2=None, op0=AL.mult)
                dm = bsp.tile([P, P], F16, name=f"dm_{i}_{j}", tag="bsc")
                nc.vector.scalar_tensor_tensor(
                    out=dm[:], in0=band_ap(BD_DYN(i, 2)),
                    scalar=aA[:, 2 * ns + j:2 * ns + j + 1], in1=t5[:],
                    op0=AL.mult, op1=AL.add)
                for ci, (c0, c1) in enumerate(CHUNKS):
                    fillers.append((ci, c0, c1, dm, r[j]))
            fillers.reverse()

            def emit_fillers(k):
                for _ in range(min(k, len(fillers))):
                    ci, c0, c1, bnd, rhs = fillers.pop()
                    mm_chunk(ci, c0, c1, bnd[:], rhs[:, c0:c1])

            # ---- u1 -> r1 -> u2#1 pipelined phase ----
            # ACT: r1 relu evacs; DVE: masked sumsq of u2 (cols 0:SUB only)
            r1t = {}
            R1m = acc.tile([P, 4 * ns], F32, name=f"R1m{i}", tag="R1m")
            SQB = acc.tile([P, 2 * ns], F32, name=f"SQB{i}", tag="SQB")
            for j in range(ns):
                for op in range(2):
                    col = op * ns + j
                    r1 = r1p.tile([P, F], F16, name=f"r1_{i}_{j}_{op}", tag="r1")
                    for hf, (h0, h1) in enumerate(HALVES):
                        cp = cps.tile([P, 1096], F32, name="cpT", tag="cps")
                        for (c0, c1) in CH0:
                            nc.tensor.matmul(cp[:, c0:c1], band_ap(BD_DYN(i, op)),
                                             r[j][:, h0 + c0:h0 + c1],
                                             start=True, stop=True)
                        emit_fillers(2)
                        nc.scalar.activation(
                            r1[:, h0:h1], cp[:, 0:h1 - h0], AF.Relu,
                            bias=bA[:, col:col + 1], scale=aA[:, col:col + 1],
                            accum_out=R1m[:, 2 * col + hf:2 * col + hf + 1])
                    nc.vector.memset(r1[0:4, 0:8], 0.0)
                    # only rows 12:16 of the tail can leak into kept outputs
                    nc.sync.dma_start(r1[12:16, 2184:2192], zeros_sb[0:4, 0:8])
                    r1t[(j, op)] = r1
                    # u2 stats conv on first SUB columns only (one chunk)
                    cp = cps.tile([P, 1096], F32, name="cpT", tag="cps")
                    nc.tensor.matmul(cp[:, 0:SUB], band_ap(BD_DYN(i, 4 + op)),
                                     r1[:, 0:SUB], start=True, stop=True)
                    emit_fillers(2)
                    # masked sumsq on DVE: copy to SBUF then STT
                    tq = trp.tile([P, SUB], F16, name="tq", tag="trash")
                    nc.vector.tensor_copy(tq[:], cp[:, 0:SUB])
                    tq2 = trp.tile([P, SUB], F16, name="tq2", tag="trash")
                    nc.vector.scalar_tensor_tensor(
                        out=tq2[:], in0=tq[:], scalar=mask_ap, in1=tq[:],
                        op0=AL.mult, op1=AL.mult,
                        accum_out=SQB[:, col:col + 1])

            emit_fillers(len(fillers))

            # ---- round B sums + allreduce ----
            R1s = acc.tile([P, 2 * ns], F32, name=f"R1s{i}", tag="R1s")
            nc.vector.tensor_tensor(out=R1s[:], in0=R1m[:, 0:4 * ns:2],
                                    in1=R1m[:, 1:4 * ns:2], op=AL.add)
            psb = cps.tile([1, 2 * ns], F32, name=f"psb{i}", tag="cps")
            for op in range(2):
                nc.tensor.matmul(psb[:, op * ns:(op + 1) * ns],
                                 cview("cmat2", 8)[:, 2 * i + op:2 * i + op + 1],
                                 R1s[:, op * ns:(op + 1) * ns],
                                 start=True, stop=True)
            redb = acc.tile([P, 2 * ns], F32, name=f"redb{i}", tag="redb")
            nc.gpsimd.partition_all_reduce(redb[:], SQB[:], channels=P,
                                           reduce_op=bass_isa.ReduceOp.add)
            brow = acc.tile([1, 4 * ns], F32, name=f"brow{i}", tag="brow")
            nc.vector.tensor_copy(brow[0:1, 0:2 * ns], redb[0:1, 0:2 * ns])
            nc.vector.tensor_copy(brow[0:1, 2 * ns:4 * ns], psb[:])
            ar_in = drp.tile([1, 4 * ns], F32, name=f"arB{i}", tag="arB")
            ar_out = drp.tile([1, 4 * ns], F32, name=f"aroB{i}", tag="aroB",
                              addr_space="Shared")
            nc.gpsimd.dma_start(ar_in[:], brow[:])
            nc.gpsimd.collective_compute(
                "AllReduce", AL.add, replica_groups=[list(range(NCORE))],
                ins=[ar_in.opt()], outs=[ar_out.opt()])
            bb = acc.tile([P, 4 * ns], F32, name=f"bb{i}", tag="bb")
            bbr = acc.tile([1, 4 * ns], F32, name=f"bbr{i}", tag="bbr")
            nc.gpsimd.dma_start(bbr[:], ar_out[:])
            nc.gpsimd.partition_broadcast(bb[:], bbr[0:1, 0:4 * ns], channels=P)

            # ---- algebra B ----
            S1B = bb[:, 0:2 * ns]
            S0B = alg.tile([P, 2 * ns], F32, name=f"S0B{i}", tag="S0B")
            for op in range(2):
                rlb = alg.tile([P, ns], F32, name=f"rlb{i}_{op}", tag="rlb")
                nc.vector.tensor_scalar(out=rlb[:], in0=bA[:, op * ns:(op + 1) * ns],
                                        scalar1=0.0, scalar2=None, op0=AL.max)
                nc.vector.scalar_tensor_tensor(
                    out=S0B[:, op * ns:(op + 1) * ns], in0=rlb[:],
                    scalar=-float(K2[i, op]),
                    in1=bb[:, 2 * ns + op * ns:2 * ns + (op + 1) * ns],
                    op0=AL.mult, op1=AL.add)
            a2, b2, _ = bn_algebra(S0B, S1B, cview(("ev2", i), 2 * ns),
                                   cview(("cv2", i), 2 * ns), 2 * ns, f"B{i}",
                                   ng_sq=NGB)

            # ---- u2#2 scaled-band matmuls into s_new ----
            for j in range(ns):
                for op in range(2):
                    col = op * ns + j
                    sb2 = bsp.tile([P, P], F16, name=f"sb2_{i}_{j}_{op}", tag="bsc")
                    nc.vector.tensor_scalar(out=sb2[:], in0=band_ap(BD_DYN(i, 4 + op)),
                                            scalar1=a2[:, col:col + 1],
                                            scalar2=None, op0=AL.mult)
                    mm_full(sb2[:], r1t[(j, op)])

            # ---- beta bias for this step ----
            beta = alg.tile([P, 2], F32, name=f"beta{i}", tag="beta")
            nc.vector.tensor_reduce(out=beta[:, 0:1], in_=bA[:, 2 * ns:4 * ns],
                                    axis=mybir.AxisListType.X, op=AL.add)
            nc.vector.tensor_reduce(out=beta[:, 1:2], in_=b2[:, 0:2 * ns],
                                    axis=mybir.AxisListType.X, op=AL.add)
            betas = alg.tile([P, 1], F32, name=f"betas{i}", tag="betas")
            nc.vector.tensor_tensor(out=betas[:], in0=beta[:, 0:1],
                                    in1=beta[:, 1:2], op=AL.add)

            last_step = (i == dbg_steps - 1)
            if last_step and os.environ.get("DBGDUMP"):
                dbgt = sbp.tile([P, 200], F32, name="dbgt", tag="dbgt")
                nc.vector.memset(dbgt[:], 0.0)
                nc.vector.tensor_copy(dbgt[:, 0:4 * NLAGX], rho_g[:, 0:4 * NLAGX])
                nc.vector.tensor_copy(dbgt[:, 40:40 + 64], s0a_g[:, 0:64])
                nc.vector.tensor_copy(dbgt[:, 110:110 + 4 * ns], aA[:])
                nc.vector.tensor_copy(dbgt[:, 130:130 + 4 * ns], bA[:])
                nc.vector.tensor_copy(dbgt[:, 150:150 + 2 * ns], a2[:])
                nc.vector.tensor_copy(dbgt[:, 170:170 + 2 * ns], b2[:])
                nc.gpsimd.dma_start(dbgd.ap(), dbgt[:])
            if not last_step:
                hn = h[i + 1]
                nc.scalar.activation(hn[:], sps[:], AF.Identity, bias=betas[:, 0:1])
                nc.vector.memset(hn[0:4, 0:8], 0.0)
                nc.gpsimd.dma_start(stash[0:8, 0:8], hn[4:12, 2184:2192])
                nc.vector.memset(hn[:, 2184:2192], 0.0)
                nc.gpsimd.dma_start(hn[4:12, 2184:2192], stash[0:8, 0:8])
                nc.gpsimd.dma_start(hn[0:4, 8:2192], hn[120:124, 0:2184])
                nc.gpsimd.dma_start(hn[124:128, 0:2184], hn[4:8, 8:2192])
                state_birth(i + 1)
                emit_maxpool(i + 1)
            else:
                if dbg_steps == STEPS:
                    # fold 0.25*(h1+h2+h3) into psum, evac scaled
                    for shist in range(1, STEPS):
                        mm_full(band_ap(BD_I), h[shist])
                    bq = alg.tile([P, 1], F32, name="bq", tag="betas")
                    nc.vector.tensor_scalar(out=bq[:], in0=betas[:], scalar1=0.25,
                                            scalar2=None, op0=AL.mult)
                    fin = sbp.tile([P, F], F32, name="fin", tag="fin")
                    nc.scalar.activation(fin[:], sps[:], AF.Identity,
                                         bias=bq[:, 0:1], scale=0.25)
                    nc.gpsimd.dma_start(out_d.ap(), fin[:])
                else:
                    # debug: emit h[i+1]-equivalent directly as fp32
                    fin = sbp.tile([P, F], F32, name="fin", tag="fin")
                    nc.scalar.activation(fin[:], sps[:], AF.Identity,
                                         bias=betas[:, 0:1])
                    nc.gpsimd.dma_start(out_d.ap(), fin[:])
        ctx.close()
    nc.compile()
    return nc


def kernel(**inputs):
    hc = make_host_consts(inputs)
    nc = build_program(hc)
    x = np.asarray(inputs["x"], np.float32).reshape(B, L)
    shards = toeplitz_shard(x)
    base = {"bands": hc["bands"], "consts": hc["consts"]}
    in_maps = [dict(base, xt=shards[c]) for c in range(NCORE)]
    from concourse.bass_utils import run_bass_kernel_spmd
    res = run_bass_kernel_spmd(nc, in_maps, list(range(NCORE)))
    outs = [untoeplitz(np.asarray(res.results[c]["out"], np.float32))
            for c in range(NCORE)]
    return np.concatenate(outs, axis=0).astype(np.float32)
